# revision 50
# baseline (speedup 1.0000x reference)
"""AttentionRNN Trainium2 kernel.

Data-parallel across 8 NeuronCores on the batch axis (B=8 -> 1 sequence per
core). Everything (embedding gather, input projection, sequential RNN scan,
additive attention, output projection) runs on-device; the host only reshapes
and shards inputs and reassembles the output.

Self-contained: hardcodes all shapes; reads nothing from disk.
"""

import sys

sys.path.insert(0, "/opt/trn_rl_repo")

import numpy as np

import concourse.bacc as bacc
import concourse.mybir as mybir
import concourse.tile as tile
from concourse.bass import IndirectOffsetOnAxis
from concourse.bass_utils import run_bass_kernel_spmd
from concourse.masks import make_identity

V, E, H, B, T = 32000, 256, 256, 8, 256
P = 128
NCORE = 8
F32 = mybir.dt.float32
F32R = mybir.dt.float32r
I32 = mybir.dt.int32
AF = mybir.ActivationFunctionType
BF16 = mybir.dt.bfloat16
FP16 = mybir.dt.float16
SCAN_NP = np.float16  # numpy dtype fed to the whh input
CH = 8  # t-chunk size in the attention energy loop
NS = 1000  # free-dim slab for the output projection stream
SUB = 500  # matmul free-dim sub-chunk (one PSUM bank)
NPRE = 15  # W_out slabs prefetched from kernel start


def _r2(w):
    """[2P, M] -> [P, 2, M] with w2[p, k, m] = w[k*P+p, m]"""
    return np.ascontiguousarray(w.reshape(2, P, -1).transpose(1, 0, 2))


def _col(b):
    """[2P] -> [P, 2] with c[p, k] = b[k*P+p]"""
    return np.ascontiguousarray(b.reshape(2, P).T)


def build_nc(dbg=False, zb=False, scan_dt=FP16):
    nc = bacc.Bacc("TRN2", target_bir_lowering=False, debug=False)

    idx_d = nc.dram_tensor("idx", [P, 2], I32, kind="ExternalInput")
    emb_d = nc.dram_tensor("emb", [V, E], F32, kind="ExternalInput")
    wih_d = nc.dram_tensor("wih", [P, 2, H], F32, kind="ExternalInput")
    whh_d = nc.dram_tensor("whh", [P, 2, H], scan_dt, kind="ExternalInput")
    wac_d = nc.dram_tensor("wac", [P, 2, H], FP16, kind="ExternalInput")
    wap_d = nc.dram_tensor("wap", [P, 2, H], FP16, kind="ExternalInput")
    bih_d = nc.dram_tensor("bih", [P, 2], F32, kind="ExternalInput")
    bhh_d = nc.dram_tensor("bhh", [P, 2], F32, kind="ExternalInput")
    bac_d = nc.dram_tensor("bac", [P, 2], F32, kind="ExternalInput")
    bap_d = nc.dram_tensor("bap", [P, 2], F32, kind="ExternalInput")
    v_d = nc.dram_tensor("vcol", [P, 2], FP16, kind="ExternalInput")
    mask_d = nc.dram_tensor("maskadd", [P, 2, T], F32, kind="ExternalInput")
    wout_d = nc.dram_tensor("wout", [P, 4, V], BF16, kind="ExternalInput")
    bout_d = nc.dram_tensor("bout", [1, V], BF16, kind="ExternalInput")
    ones_d = nc.dram_tensor("ones", [1, P], BF16, kind="ExternalInput")
    out_d = nc.dram_tensor("out", [T, V], FP16, kind="ExternalOutput")
    if dbg:
        dbg_scores = nc.dram_tensor("dbg_scores", [P, 2, T], F32, kind="ExternalOutput")
        dbg_alpha = nc.dram_tensor("dbg_alpha", [P, 2, T], F32, kind="ExternalOutput")
        dbg_comb = nc.dram_tensor("dbg_comb", [P, 4, T], F32, kind="ExternalOutput")
        dbg_qt = nc.dram_tensor("dbg_qt", [P, 2, T], F32, kind="ExternalOutput")

    with tile.TileContext(nc) as tc:
        with tc.tile_pool(name="persist", bufs=1) as pp:
            # --- persistent SBUF state ---
            idx_sb = pp.tile([P, 2], I32)
            wih = pp.tile([P, 2, H], F32)
            whh = pp.tile([P, 2, H], scan_dt)
            wac = pp.tile([P, 2, H], FP16)
            wap = pp.tile([P, 2, H], FP16)
            bih = pp.tile([P, 2], F32)
            bhh = pp.tile([P, 2], F32)
            bac = pp.tile([P, 2], F32)
            bap = pp.tile([P, 2], F32)
            vcol = pp.tile([P, 2], FP16)
            maskadd = pp.tile([P, 2, T], F32)
            ident = pp.tile([P, P], F32)
            ones_row = pp.tile([1, P], BF16)
            zeros_col = pp.tile([P, 1], F32)
            bx = pp.tile([P, 2], F32)
            embT = pp.tile([P, 2, T], F32)  # [e_p, et, t]
            xT = pp.tile([P, 2, T], F32)  # [h_p, ht, t] = x_proj^T + b_ih + b_hh
            combT = pp.tile([P, 4, T], FP16)  # [:,0:2]=context^T, [:,2:4]=Hs^T
            ident16 = pp.tile([P, P], FP16)
            qT = pp.tile([P, 2, T], F32)
            scores = pp.tile([P, 2, T], F32)  # [tp, tc, j], t = tc*128+tp
            ssum = pp.tile([P, 2], F32)
            srecip = pp.tile([P, 2], F32)
            alphaT = pp.tile([P, 2, T], FP16)  # [j_p, jt, t]
            hs = pp.tile([P, 2, H], FP16)  # [t_p, tc, h] (Hs, untransposed)
            combTr = pp.tile([P, 4, T], BF16)  # bf16 copy for the out-proj
            kTb = pp.tile([P, 2, T], FP16)  # fp16 copy of kT for the energy adds

            nc.sync.dma_start(idx_sb[:], idx_d[:])
            nc.sync.dma_start(wih[:], wih_d[:])
            nc.sync.dma_start(whh[:], whh_d[:])
            nc.sync.dma_start(wac[:], wac_d[:])
            nc.sync.dma_start(wap[:], wap_d[:])
            nc.sync.dma_start(bih[:], bih_d[:])
            nc.sync.dma_start(bhh[:], bhh_d[:])
            nc.sync.dma_start(bac[:], bac_d[:])
            nc.sync.dma_start(bap[:], bap_d[:])
            nc.sync.dma_start(vcol[:], v_d[:])
            nc.sync.dma_start(maskadd[:], mask_d[:])
            make_identity(nc, ident[:])
            nc.vector.tensor_copy(ident16[:], ident[:])
            nc.sync.dma_start(ones_row[:], ones_d[:])
            nc.gpsimd.memset(zeros_col[:], 0.0)
            nc.vector.tensor_add(bx[:], bih[:], bhh[:])

            # W_out slab prefetch pool: created first so slab DMAs stream
            # during the scan/attention phases.
            wp_cm = tc.tile_pool(name="wpool", bufs=NPRE)
            wp = wp_cm.__enter__()
            wsl_tiles = []
            for s_ in range(NPRE):
                wsl = wp.tile([P, 4, NS], BF16, tag="wslab")
                nc.sync.dma_start(wsl[:], wout_d[:, :, s_ * NS : (s_ + 1) * NS])
                wsl_tiles.append(wsl)

            # --- phase A: embedding gather + transpose to embT [e, t] ---
            with (
                tc.tile_pool(name="pha", bufs=2) as pa,
                tc.tile_pool(name="pha_ps", bufs=2, space="PSUM") as pa_ps,
            ):
                for c in range(2):
                    emb_g = pa.tile([P, E], F32, tag="embg")
                    nc.gpsimd.indirect_dma_start(
                        out=emb_g[:],
                        out_offset=None,
                        in_=emb_d[:, :],
                        in_offset=IndirectOffsetOnAxis(ap=idx_sb[:, c : c + 1], axis=0),
                    )
                    for et in range(2):
                        tr_ps = pa_ps.tile([P, P], F32, tag="trps")
                        nc.tensor.transpose(
                            tr_ps[:], emb_g[:, et * P : (et + 1) * P], ident[:]
                        )
                        nc.vector.tensor_copy(
                            embT[:, et, c * P : (c + 1) * P], tr_ps[:]
                        )

                # --- phase B: xT = (emb @ W_ih)^T + b_ih + b_hh ---
                for mt in range(2):
                    ps = pa_ps.tile([P, T], F32, tag="projps")
                    for kt in range(2):
                        nc.tensor.matmul(
                            ps[:],
                            wih[:, kt, mt * P : (mt + 1) * P],
                            embT[:, kt, :],
                            start=(kt == 0),
                            stop=(kt == 1),
                        )
                    if zb:
                        nc.scalar.activation(xT[:, mt, :], ps[:], AF.Copy)
                    else:
                        nc.scalar.activation(
                            xT[:, mt, :], ps[:], AF.Identity, bias=bx[:, mt : mt + 1]
                        )

            # --- phases C+D+E fused: scan with attention pipelined under it ---
            # The scan is ACT-latency-bound; the attention chunk work (DVE
            # energy adds, PE v-reduce matmuls, gpsimd scatter DMAs) rides on
            # the other engines\' slack. Chunk c needs Hs/q/k columns only up
            # to t=8c+8, so it is emitted 4 chunks behind the scan front.
            n_chunks = T // CH
            with (
                tc.tile_pool(name="scan_ps", bufs=1, space="PSUM") as sc_ps,
                tc.tile_pool(name="qk_ps", bufs=2, space="PSUM") as qk_ps,
                tc.tile_pool(name="epool", bufs=3) as ep,
                tc.tile_pool(name="rowpool", bufs=2) as rp,
                tc.tile_pool(name="scrpool", bufs=4, space="DRAM") as scrp,
                tc.tile_pool(name="eps", bufs=1, space="PSUM") as e_ps,
            ):

                def emit_qk_block(b):
                    cols = slice(32 * b, 32 * b + 32)
                    qp = qk_ps.tile([P, 4, 32], F32, tag="qkps", name=f"qkps{b}")
                    for wi, w_sb in enumerate((wac, wap)):
                        for mt in range(2):
                            g = 2 * wi + mt
                            for kt in range(2):
                                nc.tensor.matmul(
                                    qp[:, g, :],
                                    w_sb[:, kt, mt * P : (mt + 1) * P],
                                    combT[:, 2 + kt, cols],
                                    start=(kt == 0),
                                    stop=(kt == 1),
                                )
                    for mt in range(2):
                        if zb:
                            nc.vector.tensor_copy(qT[:, mt, cols], qp[:, mt, :])
                            nc.vector.tensor_copy(kTb[:, mt, cols], qp[:, 2 + mt, :])
                        else:
                            nc.scalar.activation(
                                qT[:, mt, cols],
                                qp[:, mt, :],
                                AF.Identity,
                                bias=bac[:, mt : mt + 1],
                            )
                            nc.scalar.activation(
                                kTb[:, mt, cols],
                                qp[:, 2 + mt, :],
                                AF.Identity,
                                bias=bap[:, mt : mt + 1],
                            )

                def emit_chunk(c):
                    t0 = c * CH
                    jcap = min(T, t0 + CH)
                    w = CH * jcap
                    et = ep.tile([P, 2, CH * T], FP16, tag="etile", name=f"et{c}")
                    for kt in range(2):
                        for tl in range(CH):
                            nc.vector.tensor_scalar_add(
                                et[:, kt, tl * jcap : (tl + 1) * jcap],
                                kTb[:, kt, :jcap],
                                qT[:, kt, t0 + tl : t0 + tl + 1],
                            )
                    nc.scalar.activation(et[:, :, :w], et[:, :, :w], AF.Tanh)
                    ps = e_ps.tile([1, CH // 2, 512], F32, tag="spsum", name=f"sps{c}")
                    for i in range(CH // 2):
                        sl = slice(2 * i * jcap, (2 * i + 2) * jcap)
                        for kt in range(2):
                            nc.tensor.matmul(
                                ps[0:1, i, : 2 * jcap],
                                vcol[:, kt : kt + 1],
                                et[:, kt, sl],
                                start=(kt == 0),
                                stop=(kt == 1),
                            )
                    row = rp.tile([1, CH * T], F32, tag="rowtile", name=f"rw{c}")
                    rview = row[:, :w].rearrange("p (i x) -> p i x", x=2 * jcap)
                    nc.vector.tensor_copy(rview, ps[:, :, : 2 * jcap])
                    tc_i = t0 // P
                    tp0 = t0 % P
                    scr = scrp.tile([CH, T], F32, tag="scr", name=f"scr{c}")
                    nc.gpsimd.dma_start(
                        scr[:, 0:jcap],
                        row[0:1, :w].rearrange("p (t j) -> p t j", j=jcap),
                    )
                    nc.gpsimd.dma_start(
                        scores[tp0 : tp0 + CH, tc_i, 0:jcap], scr[:, 0:jcap]
                    )

                def emit_hs_half(tc_i):
                    for ht in range(2):
                        tr_ps = qk_ps.tile(
                            [P, P], FP16, tag="qkps", name=f"hst{tc_i}{ht}"
                        )
                        nc.tensor.transpose(
                            tr_ps[:],
                            combT[:, 2 + ht, tc_i * P : (tc_i + 1) * P],
                            ident16[:],
                        )
                        nc.vector.tensor_copy(
                            hs[:, tc_i, ht * P : (ht + 1) * P], tr_ps[:]
                        )

                def emit_softmax_half(tc_i):
                    sl = scores[:, tc_i, :]
                    nc.vector.tensor_tensor(
                        sl, sl, maskadd[:, tc_i, :], mybir.AluOpType.add
                    )
                    nc.scalar.activation(sl, sl, AF.Exp)
                    nc.vector.reduce_sum(
                        ssum[:, tc_i : tc_i + 1], sl, axis=mybir.AxisListType.X
                    )
                    nc.vector.reciprocal(
                        srecip[:, tc_i : tc_i + 1], ssum[:, tc_i : tc_i + 1]
                    )
                    nc.vector.tensor_tensor(
                        sl,
                        sl,
                        srecip[:, tc_i : tc_i + 1].to_broadcast([P, T]),
                        mybir.AluOpType.mult,
                    )
                    for jt in range(2):
                        tr_ps = qk_ps.tile(
                            [P, P], F32, tag="qkps", name=f"atr{tc_i}{jt}"
                        )
                        nc.tensor.transpose(
                            tr_ps[:], scores[:, tc_i, jt * P : (jt + 1) * P], ident[:]
                        )
                        nc.vector.tensor_copy(
                            alphaT[:, jt, tc_i * P : (tc_i + 1) * P], tr_ps[:]
                        )

                nc.gpsimd.memset(scores[:], 0.0)
                for mt in range(2):
                    nc.scalar.activation(
                        combT[:, 2 + mt, 0:1],
                        zeros_col[:],
                        AF.Tanh,
                        bias=xT[:, mt, 0:1],
                    )
                for t in range(1, T):
                    for mt in range(2):
                        ps = sc_ps.tile([P, 1], F32, tag=f"scanps{mt}", name=f"sc{t}_{mt}")
                        for kt in range(2):
                            nc.tensor.matmul(
                                ps[:],
                                whh[:, kt, mt * P : (mt + 1) * P],
                                combT[:, 2 + kt, t - 1 : t],
                                start=(kt == 0),
                                stop=(kt == 1),
                            )
                        nc.scalar.activation(
                            combT[:, 2 + mt, t : t + 1],
                            ps[:],
                            AF.Tanh,
                            bias=xT[:, mt, t : t + 1],
                        )
                    if t % 32 == 31:
                        emit_qk_block(t // 32)
                    if t % CH == CH - 1 and t >= 39:
                        emit_chunk((t - 39) // CH)
                    if t == 127:
                        emit_hs_half(0)
                    if t == 175:
                        emit_softmax_half(0)
                emit_hs_half(1)
                for c in range((T - 39) // CH + 1, n_chunks):
                    emit_chunk(c)
                emit_softmax_half(1)

            # --- phase F tail: context matmuls + combined assembly ---
            with tc.tile_pool(name="pf_ps", bufs=2, space="PSUM") as pf_ps:
                if dbg:
                    nc.sync.dma_start(dbg_scores[:], scores[:])
                    nc.sync.dma_start(dbg_qt[:], qT[:])
                for tc_i in range(2):
                    # context^T[h, tc-half] = Hs^T @ alpha^T  (lhsT = Hs[j, h])
                    for mt in range(2):
                        ps = pf_ps.tile([P, P], F32, tag="ctxps")
                        for jt in range(2):
                            nc.tensor.matmul(
                                ps[:],
                                hs[:, jt, mt * P : (mt + 1) * P],
                                alphaT[:, jt, tc_i * P : (tc_i + 1) * P],
                                start=(jt == 0),
                                stop=(jt == 1),
                            )
                        nc.scalar.activation(
                            combT[:, mt, tc_i * P : (tc_i + 1) * P], ps[:], AF.Copy
                        )
                    if tc_i == 0:
                        # t=0 has no past: zero the context column
                        nc.gpsimd.memset(combT[:, 0:2, 0:1], 0.0)
                    nc.vector.tensor_copy(
                        combTr[:, :, tc_i * P : (tc_i + 1) * P],
                        combT[:, :, tc_i * P : (tc_i + 1) * P],
                    )
                if dbg:
                    nc.sync.dma_start(dbg_alpha[:], scores[:])
                    nc.gpsimd.dma_start(dbg_comb[:], combT[:])

            # --- phase G: out = combined @ W_out + b_out ---
            n_slabs = V // NS
            with (
                tc.tile_pool(name="bpool", bufs=2) as bp,
                tc.tile_pool(name="opool", bufs=2) as op,
                tc.tile_pool(name="pg_ps", bufs=2, space="PSUM") as pg_ps,
            ):
                for s in range(n_slabs):
                    n0 = s * NS
                    if s < NPRE:
                        wsl = wsl_tiles[s]
                    else:
                        wsl = wp.tile([P, 4, NS], BF16, tag="wslab")
                        nc.sync.dma_start(wsl[:], wout_d[:, :, n0 : n0 + NS])
                    if not zb:
                        bsl = bp.tile([1, NS], BF16, tag="bslab")
                        nc.sync.dma_start(bsl[:], bout_d[:, n0 : n0 + NS])
                    nsub = NS // SUB
                    for mt in range(2):
                        osb = op.tile([P, NS], FP16, tag=f"osb{mt}")
                        pss = [
                            pg_ps.tile(
                                [P, SUB], F32, tag=f"ops{mt}{i}", name=f"ops{mt}{i}"
                            )
                            for i in range(nsub)
                        ]
                        for kt in range(4):
                            for i in range(nsub):
                                nc.tensor.matmul(
                                    pss[i][:],
                                    combTr[:, kt, mt * P : (mt + 1) * P],
                                    wsl[:, kt, i * SUB : (i + 1) * SUB],
                                    start=(kt == 0),
                                    stop=(zb and kt == 3),
                                )
                        if not zb:
                            for i in range(nsub):
                                nc.tensor.matmul(
                                    pss[i][:],
                                    ones_row[:],
                                    bsl[:, i * SUB : (i + 1) * SUB],
                                    start=False,
                                    stop=True,
                                )
                        for i in range(nsub):
                            if mt == 1 and s % 2 == 1:
                                nc.vector.tensor_copy(
                                    osb[:, i * SUB : (i + 1) * SUB], pss[i][:]
                                )
                            else:
                                nc.scalar.activation(
                                    osb[:, i * SUB : (i + 1) * SUB],
                                    pss[i][:],
                                    AF.Copy,
                                )
                        nc.sync.dma_start(
                            out_d[mt * P : (mt + 1) * P, n0 : n0 + NS], osb[:]
                        )
            wp_cm.__exit__(None, None, None)

    nc.compile()
    return nc


_NC_CACHE = {}


def _get_nc(zb):
    key = ("nc", zb)
    if key not in _NC_CACHE:
        _NC_CACHE[key] = build_nc(zb=zb)
    return _NC_CACHE[key]


def _prep(inputs):
    input = np.asarray(inputs["input"])
    embedding = np.ascontiguousarray(np.asarray(inputs["embedding"], np.float32))
    W_ih, b_ih = inputs["W_ih"], inputs["b_ih"]
    W_hh, b_hh = inputs["W_hh"], inputs["b_hh"]
    W_ac, b_ac = inputs["W_ac"], inputs["b_ac"]
    W_ap, b_ap = inputs["W_ap"], inputs["b_ap"]
    v_attn, W_out, b_out = inputs["v_attn"], inputs["W_out"], inputs["b_out"]
    zb = bool(
        not np.any(b_ih)
        and not np.any(b_hh)
        and not np.any(b_ac)
        and not np.any(b_ap)
        and not np.any(b_out)
    )

    t_idx = np.arange(T)
    j_idx = np.arange(T)
    maskadd = np.where(
        j_idx[None, :] < (t_idx[:, None]), 0.0, -1e9
    ).astype(np.float32)  # [t, j]
    maskadd = np.ascontiguousarray(
        maskadd.reshape(2, P, T).transpose(1, 0, 2)
    )  # [tp, tc, j]

    import ml_dtypes

    wout_r = np.ascontiguousarray(
        np.asarray(W_out, np.float32)
        .astype(ml_dtypes.bfloat16)
        .reshape(4, P, V)
        .transpose(1, 0, 2)
    )
    shared = {
        "emb": embedding,
        "wih": _r2(np.asarray(W_ih, np.float32)),
        "whh": _r2(np.asarray(W_hh, np.float32).astype(SCAN_NP)),
        "wac": _r2(np.asarray(W_ac, np.float32).astype(np.float16)),
        "wap": _r2(np.asarray(W_ap, np.float32).astype(np.float16)),
        "bih": _col(np.asarray(b_ih, np.float32)),
        "bhh": _col(np.asarray(b_hh, np.float32)),
        "bac": _col(np.asarray(b_ac, np.float32)),
        "bap": _col(np.asarray(b_ap, np.float32)),
        "vcol": _col(np.asarray(v_attn, np.float32).astype(np.float16)),
        "maskadd": maskadd,
        "wout": wout_r,
        "bout": np.ascontiguousarray(
            np.asarray(b_out, np.float32).astype(ml_dtypes.bfloat16)[None, :]
        ),
        "ones": np.ones((1, P), ml_dtypes.bfloat16),
    }
    in_maps = []
    for b in range(B):
        m = dict(shared)
        m["idx"] = np.ascontiguousarray(
            input[b].reshape(2, P).T.astype(np.int32)
        )
        in_maps.append(m)

    return in_maps, zb


def _run(inputs, trace=False):
    in_maps, zb = _prep(inputs)
    nc = _get_nc(zb)
    res = run_bass_kernel_spmd(nc, in_maps, list(range(NCORE)), trace=trace)
    out = np.stack([res.results[c]["out"] for c in range(NCORE)], axis=0)
    return np.ascontiguousarray(out.astype(np.float32)), res.exec_time_ns


def kernel(**inputs):
    return _run(inputs)[0]
